# revision 6
# baseline (speedup 1.0000x reference)
"""MLA (multi-head latent attention) forward kernel for Trainium2, 8 NeuronCores.

Sharding v2: data-parallel over batch (B=2) x 4-way within each batch:
the x->qa / x->ckv low-rank projections are TOKEN-sharded (each core
computes its 512-token chunk, applies RMSNorm locally, then the scaled
qa/ckv/rope activations are AllGather'ed in bf16 across the 4 cores of
the batch group). Everything downstream (q_b/kv_b projections,
attention, o_proj) is head-sharded exactly like v1: core c handles
batch c//4, head-group c%4; the host sums the 4 head-group o_proj
partials per batch.

This removes the 4x replication of the A-phase GEMMs (46% of v1's PE
cycles). All matmuls are pure bf16 x bf16 (1 PE cycle/row at any free
size; fp32 PSUM accumulate), which also removes the f32r small-free
4x penalty on causal-diagonal attention tiles.

Structure per core:
  A:  ckv chains (4) + rope (1) -> rmsnorm(local) -> scale -> bounce ->
      AllGather g_b (fires ~20% into A);
      qa chains (12) -> rmsnorm(local) -> scale -> bounce -> AllGather g_a.
      Sum-of-squares via ones-matmul chains software-pipelined behind
      the A matmuls.
  Bkv: per gathered 512-token pass: kn^T per head -> SBUF resident;
      v rows -> SBUF resident.
  Bq: per pass: qn^T/qr^T -> SBUF resident (no DRAM roundtrip).
  Attention per (head, 512-wide tq chunk) in S^T layout, causal:
      S^T = kn-tile.T @ qn + kpe-pad-tile.T @ qr-pair (rope zero-padded
      to K=128). P^T = exp(S^T*SCALE) in bf16 feeds AV directly.
      Column sums l via ones-matmul chain; O^T scaled by 1/l.
  o_proj: out[tq,:] = sum_h O^T[h].T @ WoT[h] -> DMA to DRAM.
"""

import sys

if "/opt/trn_rl_repo" not in sys.path:
    sys.path.insert(0, "/opt/trn_rl_repo")

import numpy as np

import concourse.bass as bass
import concourse.mybir as mybir
from concourse import bacc
from concourse.tile import TileContext

F32 = mybir.dt.float32
BF16 = mybir.dt.bfloat16

B, T, C = 2, 2048, 2048
H, HG = 16, 4  # total heads, heads per core
QL = 1536      # q lora
KVL = 512      # kv lora
ROPE = 64
NOPE = 128
QHD = NOPE + ROPE  # 192
VHD = 128
EPS = 1e-6
SCALE = QHD ** -0.5
MASK_VAL = -1e9  # added pre-scale; exp((s+MASK_VAL)*SCALE) == 0.0

NT = T // 128        # 16 t tiles
NC_TILES = C // 128  # 16 contraction tiles over C
NJQ = QL // 128      # 12
NJK = KVL // 128     # 4
TCH = 512            # tokens per core chunk (A-phase shard)
GROUPS = [[0, 1, 2, 3], [4, 5, 6, 7]]


def make_causal_mask_T(nc, mask, mask_val):
    """Additive mask for S^T tiles: keep (0) where col >= row, else mask_val."""
    nc.gpsimd.memset(mask, 0.0)
    nc.gpsimd.affine_select(
        out=mask,
        in_=mask,
        compare_op=mybir.AluOpType.is_ge,
        fill=mask_val,
        base=0,
        pattern=[[1, mask.shape[1]]],
        channel_multiplier=-1,
    )


def build_program() -> bass.Bass:
    nc = bacc.Bacc(num_devices=8)

    xT_chunk = nc.dram_tensor("xT_chunk", [C, TCH], BF16, kind="ExternalInput")
    wqa_pk = nc.dram_tensor("wqa_pk", [NJQ, 128, NC_TILES, 128], BF16, kind="ExternalInput")
    wkva_pk = nc.dram_tensor("wkva_pk", [NJK, 128, NC_TILES, 128], BF16, kind="ExternalInput")
    wrope_pk = nc.dram_tensor("wrope_pk", [128, NC_TILES, 64], BF16, kind="ExternalInput")
    wqbT_n = nc.dram_tensor("wqbT_n", [QL, HG * NOPE], BF16, kind="ExternalInput")
    wqbT_r = nc.dram_tensor("wqbT_r", [QL, 2 * 128], BF16, kind="ExternalInput")
    wkvbT_n = nc.dram_tensor("wkvbT_n", [KVL, HG * NOPE], BF16, kind="ExternalInput")
    wkvbT_v = nc.dram_tensor("wkvbT_v", [KVL, HG * VHD], BF16, kind="ExternalInput")
    woT = nc.dram_tensor("woT", [128, HG * C], BF16, kind="ExternalInput")
    out = nc.dram_tensor("out", [T, C], F32, kind="ExternalOutput")

    with TileContext(nc) as tc:
        with tc.tile_pool(name="dram", bufs=1, space="DRAM") as dram_pool:
            bounce_a = dram_pool.tile([NJQ, 128, TCH], BF16)
            bounce_b = dram_pool.tile([NJK + 1, 128, TCH], BF16)
            gath_a = dram_pool.tile([4, NJQ, 128, TCH], BF16)
            gath_b = dram_pool.tile([4, NJK + 1, 128, TCH], BF16)
            _build_tiled(nc, tc, locals())
    nc.finalize()
    return nc


def _build_tiled(nc, tc, io):
    xT_chunk = io["xT_chunk"]
    wqa_pk, wkva_pk, wrope_pk = io["wqa_pk"], io["wkva_pk"], io["wrope_pk"]
    wqbT_n, wqbT_r = io["wqbT_n"], io["wqbT_r"]
    wkvbT_n, wkvbT_v, woT, out = io["wkvbT_n"], io["wkvbT_v"], io["woT"], io["out"]
    bounce_a, bounce_b = io["bounce_a"], io["bounce_b"]
    gath_a, gath_b = io["gath_a"], io["gath_b"]

    from contextlib import ExitStack

    ctx = ExitStack()
    with ctx:
        # ---- small persistent constants ----
        const_pool = ctx.enter_context(tc.tile_pool(name="const", bufs=1))
        cmaskT = const_pool.tile([128, 128], F32)
        make_causal_mask_T(nc, cmaskT[:], mask_val=MASK_VAL)
        ones_stage = const_pool.tile([128, 128], F32)
        nc.vector.memset(ones_stage[:], 1.0)
        ones_bf = const_pool.tile([128, 128], BF16)
        nc.vector.tensor_copy(ones_bf[:], ones_stage[:])
        eps_t = const_pool.tile([128, 1], F32)
        nc.vector.memset(eps_t[:], EPS)
        zstage = const_pool.tile([128, 512], BF16)
        nc.vector.memset(zstage[:], 0.0)

        # ---- PE warmup (no data deps): hold the HAM un-throttled while
        # the first x / weight DMAs are in flight ----
        with tc.tile_pool(name="warm", bufs=1, space="PSUM") as wmpool:
            wm = wmpool.tile([128, 512], F32, tag="wm")
            for i in range(24):
                nc.tensor.matmul(
                    wm[:], ones_bf[:], zstage[:],
                    start=(i == 0), stop=(i == 23), skip_group_check=True,
                )

        # ---- persistent SBUF-resident activations for attention ----
        kv_pool = ctx.enter_context(tc.tile_pool(name="kv", bufs=1))
        # zero-padded rope keys: kpe_e rows 0:64 = kpe (even heads),
        # kpe_o rows 64:128 = kpe (odd heads); other half stays zero
        kpe_e = kv_pool.tile([128, T], BF16)
        kpe_o = kv_pool.tile([128, T], BF16)
        nc.vector.memset(kpe_e[:], 0.0)
        nc.vector.memset(kpe_o[:], 0.0)
        kn_sb = kv_pool.tile([128, HG, T], BF16)   # k_nope^T per head
        v_sb = kv_pool.tile([128, NT, HG * VHD], BF16)  # v rows per t-tile
        qn_sb = kv_pool.tile([128, HG, T], BF16)   # q_nope^T per head
        qr_sb = kv_pool.tile([128, 2, T], BF16)    # q_rope^T per head pair
        wo_sb = kv_pool.tile([128, HG, C], BF16)   # o_proj weights

        # resident B-phase weights (DMAs deferred into the A phase)
        res_pool = ctx.enter_context(tc.tile_pool(name="res", bufs=1))
        wqn = res_pool.tile([128, NJQ, HG * NOPE], BF16)
        wqr = res_pool.tile([128, NJQ, 256], BF16)
        wn = res_pool.tile([128, NJK, HG * NOPE], BF16)
        wv = res_pool.tile([128, NJK, HG * VHD], BF16)

        def load_resident():
            nc.scalar.dma_start(wqn[:], wqbT_n.rearrange("(j p) m -> p j m", p=128))
            nc.scalar.dma_start(wqr[:], wqbT_r.rearrange("(j p) m -> p j m", p=128))
            nc.scalar.dma_start(wn[:], wkvbT_n.rearrange("(k p) m -> p k m", p=128))
            nc.scalar.dma_start(wv[:], wkvbT_v.rearrange("(k p) m -> p k m", p=128))
            nc.scalar.dma_start(wo_sb[:], woT.rearrange("p (h c) -> p h c", c=C))

        # gathered-pass input pools + prefetch registries
        ckvs_pool = ctx.enter_context(tc.tile_pool(name="ckvs", bufs=4))
        qas_pool = ctx.enter_context(tc.tile_pool(name="qas", bufs=2))
        pf_ckv, pf_qa = {}, {}

        def prefetch_ckv(pa):
            t = ckvs_pool.tile([128, NJK, TCH], BF16, tag="ckvs")
            for kj in range(NJK):
                nc.scalar.dma_start(t[:, kj, :], gath_b[pa, kj])
            tabs = pa * TCH
            nc.scalar.dma_start(
                kpe_e[0:64, tabs:tabs + TCH], gath_b[pa, NJK, 0:64, :])
            nc.scalar.dma_start(
                kpe_o[64:128, tabs:tabs + TCH], gath_b[pa, NJK, 0:64, :])
            pf_ckv[pa] = t

        def prefetch_qa(pa):
            t = qas_pool.tile([128, NJQ, TCH], BF16, tag="qas")
            for jt in range(NJQ):
                nc.scalar.dma_start(t[:, jt, :], gath_a[pa, jt])
            pf_qa[pa] = t

        # ================= A phase: local 512-token chunk =================
        with (
            tc.tile_pool(name="p_x", bufs=1) as xpool,
            tc.tile_pool(name="p_w", bufs=3) as wpool,
            tc.tile_pool(name="p_stage", bufs=1) as stagepool,
            tc.tile_pool(name="p_sq", bufs=2) as sqpool,
            tc.tile_pool(name="p_st", bufs=1) as stpool,
            tc.tile_pool(name="p_aps", bufs=2, space="PSUM") as apsum,
            tc.tile_pool(name="p_ss", bufs=1, space="PSUM") as sspsum,
        ):
            xt = xpool.tile([128, NC_TILES, TCH], BF16, tag="xt")
            xT_r = xT_chunk.rearrange("(ct p) t -> p ct t", p=128)
            for xq in range(4):
                nc.scalar.dma_start(
                    xt[:, 4 * xq:4 * xq + 4, :],
                    xT_r[:, 4 * xq:4 * xq + 4, :],
                )

            qa_bf = stagepool.tile([128, NJQ, TCH], BF16)
            ckv_bf = stagepool.tile([128, NJK, TCH], BF16)
            rope_bf = stagepool.tile([128, TCH], BF16)
            nc.vector.memset(rope_bf[:], 0.0)

            ssq = sspsum.tile([128, TCH], F32, tag="ssq")
            ssk = sspsum.tile([128, TCH], F32, tag="ssk")

            def mk_ss(sstile, sq, sfirst, slast):
                def d():
                    nc.tensor.matmul(
                        sstile[:], ones_bf[:], sq[:],
                        start=sfirst, stop=slast, skip_group_check=True,
                    )
                return d

            def finish_ckv():
                stdk = stpool.tile([128, TCH], F32, tag="stdk")
                nc.scalar.activation(
                    stdk[:], ssk[:],
                    mybir.ActivationFunctionType.Sqrt,
                    bias=eps_t[:], scale=1.0 / KVL,
                )
                bck = stpool.tile([128, TCH], F32, tag="bck")
                nc.vector.reciprocal(bck[:], stdk[:])
                for kj in range(NJK):
                    nc.vector.tensor_mul(
                        out=ckv_bf[:, kj, :], in0=ckv_bf[:, kj, :], in1=bck[:])
                    nc.sync.dma_start(bounce_b[kj], ckv_bf[:, kj, :])
                nc.sync.dma_start(bounce_b[NJK], rope_bf[:])
                nc.gpsimd.collective_compute(
                    "AllGather", mybir.AluOpType.bypass, replica_groups=GROUPS,
                    ins=[bounce_b.opt()], outs=[gath_b.opt()],
                )
                # all 4 passes prefetched here: keeps the scalar DMA ring
                # free of ordering inversions (ckv reads all wait only on
                # g_b; qa reads emitted later wait on g_a)
                for pa in range(4):
                    prefetch_ckv(pa)

            def finish_qa():
                stdq = stpool.tile([128, TCH], F32, tag="stdq")
                nc.scalar.activation(
                    stdq[:], ssq[:],
                    mybir.ActivationFunctionType.Sqrt,
                    bias=eps_t[:], scale=1.0 / QL,
                )
                bcq = stpool.tile([128, TCH], F32, tag="bcq")
                nc.vector.reciprocal(bcq[:], stdq[:])
                for jt in range(NJQ):
                    nc.vector.tensor_mul(
                        out=qa_bf[:, jt, :], in0=qa_bf[:, jt, :], in1=bcq[:])
                    nc.sync.dma_start(bounce_a[jt], qa_bf[:, jt, :])
                nc.gpsimd.collective_compute(
                    "AllGather", mybir.AluOpType.bypass, replica_groups=GROUPS,
                    ins=[bounce_a.opt()], outs=[gath_a.opt()],
                )
                prefetch_qa(0)

            # prefetch the first two weight tiles
            wt_pf = {}

            def prefetch_wt(jt, wsrc, wcols):
                t = wpool.tile([128, NC_TILES, 128], BF16, tag="wt")
                nc.sync.dma_start(t[:, :, :wcols], wsrc)
                wt_pf[jt] = t

            def chain_src(jt):
                if jt < NJK:
                    return wkva_pk[jt], 128
                if jt == NJK:
                    return wrope_pk[:], 64
                return wqa_pk[jt - NJK - 1], 128

            prefetch_wt(0, *chain_src(0))
            prefetch_wt(1, *chain_src(1))

            deferred = None
            NCH = NJK + 1 + NJQ  # 17 chains: ckv, rope, qa
            for jt in range(NCH):
                wsrc, wcols = chain_src(jt)
                if jt in wt_pf:
                    wt = wt_pf.pop(jt)
                else:
                    wt = wpool.tile([128, NC_TILES, 128], BF16, tag="wt")
                    nc.sync.dma_start(wt[:, :, :wcols], wsrc)
                if jt + 2 < NCH:
                    prefetch_wt(jt + 2, *chain_src(jt + 2))
                ps = apsum.tile([128, TCH], F32, tag="achain")
                for ct in range(NC_TILES):
                    nc.tensor.matmul(
                        ps[:wcols],
                        wt[:, ct, :wcols],
                        xt[:, ct, :],
                        start=(ct == 0),
                        stop=(ct == NC_TILES - 1),
                    )
                if deferred is not None:
                    deferred()
                    deferred = None
                if jt == 2:
                    # before finish_ckv: must precede any collective-waiting
                    # DMA on the scalar ring (ring descriptors run in order)
                    load_resident()
                if jt < NJK:
                    sq = sqpool.tile([128, TCH], BF16, tag="sq")
                    nc.scalar.square(sq[:], ps[:])
                    deferred = mk_ss(ssk, sq, jt == 0, jt == NJK - 1)
                    nc.vector.tensor_copy(ckv_bf[:, jt, :], ps[:])
                elif jt == NJK:
                    nc.vector.tensor_copy(rope_bf[0:64, :], ps[:64])
                    finish_ckv()
                else:
                    j = jt - NJK - 1
                    sq = sqpool.tile([128, TCH], BF16, tag="sq")
                    nc.scalar.square(sq[:], ps[:])
                    deferred = mk_ss(ssq, sq, j == 0, j == NJQ - 1)
                    nc.vector.tensor_copy(qa_bf[:, j, :], ps[:])
            deferred()  # final ssq matmul
            finish_qa()

        # ================= Bkv: kn/v from gathered ckv =================
        with tc.tile_pool(name="p_kv", bufs=2, space="PSUM") as kvpsum:
            for pa in range(4):
                tabs = pa * TCH
                ckv_sb = pf_ckv.pop(pa)
                for h in range(HG):
                    ps = kvpsum.tile([128, TCH], F32, tag="kvch")
                    for kj in range(NJK):
                        nc.tensor.matmul(
                            ps[:],
                            wn[:, kj, h * NOPE:(h + 1) * NOPE],
                            ckv_sb[:, kj, :],
                            start=(kj == 0),
                            stop=(kj == NJK - 1),
                        )
                    nc.vector.tensor_copy(kn_sb[:, h, tabs:tabs + TCH], ps[:])
                for tt in range(4):
                    ps = kvpsum.tile([128, TCH], F32, tag="kvch")
                    for kj in range(NJK):
                        nc.tensor.matmul(
                            ps[:],
                            ckv_sb[:, kj, tt * 128:(tt + 1) * 128],
                            wv[:, kj, :],
                            start=(kj == 0),
                            stop=(kj == NJK - 1),
                        )
                    nc.vector.tensor_copy(v_sb[:, 4 * pa + tt, :], ps[:])

        # ================= Bq: qn/qr from gathered qa =================
        with tc.tile_pool(name="p_bq", bufs=2, space="PSUM") as bqpsum:
            for pa in range(4):
                tabs = pa * TCH
                qa_sb = pf_qa.pop(pa)
                if pa + 1 < 4:
                    prefetch_qa(pa + 1)
                for g in range(6):
                    ps = bqpsum.tile([128, TCH], F32, tag="bq")
                    for jt in range(NJQ):
                        if g < HG:
                            lhs = wqn[:, jt, g * NOPE:(g + 1) * NOPE]
                        else:
                            lhs = wqr[:, jt, (g - HG) * 128:(g - HG + 1) * 128]
                        nc.tensor.matmul(
                            ps[:],
                            lhs,
                            qa_sb[:, jt, :],
                            start=(jt == 0),
                            stop=(jt == NJQ - 1),
                        )
                    if g < HG:
                        nc.vector.tensor_copy(qn_sb[:, g, tabs:tabs + TCH], ps[:])
                    else:
                        nc.vector.tensor_copy(
                            qr_sb[:, g - HG, tabs:tabs + TCH], ps[:])

        # ================= Attention + o_proj (S^T layout) =================
        with (
            tc.tile_pool(name="at_pt", bufs=4) as ptpool,
            tc.tile_pool(name="at_st", bufs=2) as stpool,
            tc.tile_pool(name="at_ot", bufs=2) as otpool,
            tc.tile_pool(name="at_ob", bufs=4) as obpool,
            tc.tile_pool(name="at_sps", bufs=2, space="PSUM") as spsum,
            tc.tile_pool(name="at_avps", bufs=2, space="PSUM") as avpsum,
            tc.tile_pool(name="at_lps", bufs=2, space="PSUM") as lpsum,
            tc.tile_pool(name="at_ops", bufs=2, space="PSUM") as opsum,
        ):
            for c in (3, 2, 1, 0):  # 512-wide tq chunks, dense first
                q0 = c * 512
                ntk = 4 * c + 4
                ot_sb = otpool.tile([128, HG, 512], BF16, tag="ot")
                for h in range(HG):
                    qn_t = qn_sb[:, h, q0:q0 + 512]
                    qr_t = qr_sb[:, h // 2, q0:q0 + 512]
                    kpe_h = kpe_e if h % 2 == 0 else kpe_o
                    av = avpsum.tile([128, 512], F32, tag="av")
                    lch = lpsum.tile([128, 512], F32, tag="l")

                    pts, offs = [], []

                    def s_stage(j, h=h, c=c, qn_t=qn_t, qr_t=qr_t,
                                kpe_h=kpe_h):
                        off = max(0, (j - 4 * c) * 128)
                        ps = spsum.tile([128, 512], F32, tag="schain")
                        nc.tensor.matmul(
                            ps[:, off:512],
                            kn_sb[:, h, j * 128:(j + 1) * 128],
                            qn_t[:, off:512],
                            start=True,
                            stop=False,
                        )
                        nc.tensor.matmul(
                            ps[:, off:512],
                            kpe_h[:, j * 128:(j + 1) * 128],
                            qr_t[:, off:512],
                            start=False,
                            stop=True,
                        )
                        if j >= 4 * c:
                            nc.vector.tensor_add(
                                out=ps[:, off:off + 128],
                                in0=ps[:, off:off + 128],
                                in1=cmaskT[:],
                            )
                        pt = ptpool.tile([128, 512], BF16, tag="pt")
                        nc.scalar.activation(
                            pt[:, off:512],
                            ps[:, off:512],
                            mybir.ActivationFunctionType.Exp,
                            scale=SCALE,
                        )
                        pts.append(pt)
                        offs.append(off)

                    def av_stage(j, h=h, av=av, lch=lch, pts=pts, offs=offs,
                                 ntk=ntk):
                        off = offs[j]
                        nc.tensor.matmul(
                            lch[:, off:512],
                            ones_bf[:],
                            pts[j][:, off:512],
                            start=(j == 0),
                            stop=(j == ntk - 1),
                            skip_group_check=True,
                        )
                        nc.tensor.matmul(
                            av[:, off:512],
                            v_sb[:, j, h * VHD:(h + 1) * VHD],
                            pts[j][:, off:512],
                            start=(j == 0),
                            stop=(j == ntk - 1),
                            skip_group_check=True,
                        )

                    for j0 in range(min(2, ntk)):
                        s_stage(j0)
                    for j in range(ntk):
                        if j + 2 < ntk:
                            s_stage(j + 2)
                        av_stage(j)

                    linv = stpool.tile([128, 512], F32, tag="linv")
                    nc.vector.reciprocal(linv[:], lch[:])
                    nc.vector.tensor_mul(
                        out=ot_sb[:, h, :], in0=av[:], in1=linv[:]
                    )

                # o_proj for these 512 rows
                for s in range(4):
                    trow = q0 + s * 128
                    for cn in range(C // 512):
                        ps = opsum.tile([128, 512], F32, tag="oproj")
                        for h in range(HG):
                            nc.tensor.matmul(
                                ps[:],
                                ot_sb[:, h, s * 128:(s + 1) * 128],
                                wo_sb[:, h, cn * 512:(cn + 1) * 512],
                                start=(h == 0),
                                stop=(h == HG - 1),
                            )
                        osb = obpool.tile([128, 512], F32, tag="osb")
                        nc.vector.tensor_copy(osb[:], ps[:])
                        nc.sync.dma_start(
                            out[trow:trow + 128, cn * 512:(cn + 1) * 512], osb[:]
                        )


_PROGRAM_CACHE = {}


def _get_program():
    if "nc" not in _PROGRAM_CACHE:
        _PROGRAM_CACHE["nc"] = build_program()
    return _PROGRAM_CACHE["nc"]


def _shard_weights(Wqa, gqa, Wqb, Wkva, gkva, Wkvb, Wo, hg):
    import ml_dtypes
    bf16 = ml_dtypes.bfloat16
    h0 = hg * HG
    Wqb_s = (Wqb * gqa[None, :]).reshape(H, QHD, QL)
    Wn = Wqb_s[h0:h0 + HG, :NOPE, :]                    # [4,128,QL]
    Wr = Wqb_s[h0:h0 + HG, NOPE:, :]                    # [4,64,QL]
    wqbT_n = np.ascontiguousarray(Wn.reshape(HG * NOPE, QL).T)
    wqbT_r = np.ascontiguousarray(
        Wr.reshape(2, 128, QL).transpose(2, 0, 1).reshape(QL, 256))
    Wkvb_s = (Wkvb * gkva[None, :]).reshape(H, NOPE + VHD, KVL)
    wkvbT_n = np.ascontiguousarray(
        Wkvb_s[h0:h0 + HG, :NOPE, :].reshape(HG * NOPE, KVL).T)
    wkvbT_v = np.ascontiguousarray(
        Wkvb_s[h0:h0 + HG, NOPE:, :].reshape(HG * VHD, KVL).T)
    # woT packed [128, HG*C]: partition = dv, free = (h, c)
    WoT = Wo[:, h0 * VHD:(h0 + HG) * VHD].T             # [512, C]
    woT = np.ascontiguousarray(
        WoT.reshape(HG, VHD, C).transpose(1, 0, 2).reshape(VHD, HG * C))
    return {
        "wqbT_n": wqbT_n.astype(bf16),
        "wqbT_r": wqbT_r.astype(bf16),
        "wkvbT_n": wkvbT_n.astype(bf16),
        "wkvbT_v": wkvbT_v.astype(bf16),
        "woT": woT.astype(bf16),
    }


def kernel(x, Wqa, gqa, Wqb, Wkva, gkva, Wkvb, Wo):
    from concourse.bass_utils import run_bass_kernel_spmd

    x = np.asarray(x, np.float32)
    args = [np.asarray(a, np.float32) for a in (Wqa, gqa, Wqb, Wkva, gkva, Wkvb, Wo)]
    Wqa, gqa, Wqb, Wkva, gkva, Wkvb, Wo = args

    nc = _get_program()
    # pack A weights so each [128,16,128] SBUF tile is one contiguous DMA:
    # pk[jt, p, ct, col] = W[jt*128+col, ct*128+p]
    import ml_dtypes
    bf16 = ml_dtypes.bfloat16
    wqa_pk = np.ascontiguousarray(
        Wqa.reshape(NJQ, 128, NC_TILES, 128).transpose(0, 3, 2, 1)).astype(bf16)
    wkva_pk = np.ascontiguousarray(
        Wkva[:KVL].reshape(NJK, 128, NC_TILES, 128).transpose(0, 3, 2, 1)).astype(bf16)
    wrope_pk = np.ascontiguousarray(
        Wkva[KVL:].reshape(ROPE, NC_TILES, 128).transpose(2, 1, 0)).astype(bf16)
    shard_cache = [
        _shard_weights(Wqa, gqa, Wqb, Wkva, gkva, Wkvb, Wo, hg) for hg in range(4)
    ]
    xT = [np.ascontiguousarray(x[b].T).astype(bf16) for b in range(B)]

    in_maps = []
    for core in range(8):
        b, r = core // 4, core % 4
        m = {"xT_chunk": np.ascontiguousarray(xT[b][:, r * TCH:(r + 1) * TCH]),
             "wqa_pk": wqa_pk, "wkva_pk": wkva_pk, "wrope_pk": wrope_pk}
        m.update(shard_cache[r])
        in_maps.append(m)

    res = run_bass_kernel_spmd(nc, in_maps, core_ids=list(range(8)))
    out = np.zeros((B, T, C), np.float32)
    for core in range(8):
        out[core // 4] += res.results[core]["out"]
    return out
